# revision 42
# baseline (speedup 1.0000x reference)
"""Causal self-attention (B=2, T=2048, D=1024, H=16) on 8 TRN2 NeuronCores.

Sharding: data-parallel over batch (cores 0-3 -> batch 0, cores 4-7 -> batch 1),
tensor-parallel over heads (4 heads / 256 output dims per core). Each core
computes q/k/v projections for its heads, causal flash-style attention, and a
partial output projection (contraction over its 256 dims of Wo). The host sums
the 4 partials per batch and adds bo.

All matmul operands are bf16 (f32 PSUM accumulation): weight loads get FWL
(2x) and x/weight DMAs halve. The softmax exp is split across two engines:
head a of each pair runs real exp on ACT; head b runs a one-instruction
Schraudolph exp on DVE (w = alpha*S + (beta + 1.5*2^23) in f32 -- the float
add's round-to-nearest leaves round(alpha*S+beta) in the low mantissa bits,
which the PE reads as bf16 via a stride-2 bitcast view).  This halves the
exp time per k-block that paces the attention inner loop.  Softmax
normalization uses reciprocal_approx_fast straight from PSUM and a stride-0
partition-broadcast DMA instead of PE broadcast matmuls.  Output-projection
PSUM->SBUF staging runs on GpSimd so ACT only does exp.  DMA issue is spread
across the SP/ACT/Pool sequencers; outputs are staged in pairs and stored
with one DMA per 2 row-blocks.
"""
import sys

sys.path.insert(0, '/opt/trn_rl_repo')

import numpy as np

import concourse.bass as bass  # noqa: F401  (import keeps bass registered)
import concourse.mybir as mybir
import concourse.tile as tile
from concourse import bacc
from concourse.bass_utils import run_bass_kernel_spmd

F32 = mybir.dt.float32
BF16 = mybir.dt.bfloat16
AF = mybir.ActivationFunctionType
ALU = mybir.AluOpType

B, T, D, H, HD = 2, 2048, 1024, 16, 64
NCORES = 8
E = 256          # output dims per core (4 heads x 64)
DM = 8           # d_model chunks of 128
TQ = 512
NTQ = T // TQ    # 4
TKT = 128
NTKT = T // TKT  # 16

# Schraudolph exp constants: bf16 bits of exp(0.125*S) = round(ALPHA*S + BETA)
# computed via the f32 magic-add (1.5*2^23) round-to-nearest trick.
EXP_ALPHA = 0.125 * 1.4426950408889634 * 128.0    # 23.08312065422342
EXP_MAGIC = float((127 * 128 - 6) + (3 << 22))    # beta + 1.5*2^23

_CACHE = {}


def _build():
    nc = bacc.Bacc("TRN2", target_bir_lowering=False, debug=False)

    xT = nc.dram_tensor("xT", [D, T], BF16, kind="ExternalInput")
    wq = nc.dram_tensor("wq", [D, E], BF16, kind="ExternalInput")
    wk = nc.dram_tensor("wk", [D, E], BF16, kind="ExternalInput")
    wv = nc.dram_tensor("wv", [D, E], BF16, kind="ExternalInput")
    wo = nc.dram_tensor("wo", [E, D], BF16, kind="ExternalInput")
    bq_d = nc.dram_tensor("bq", [E, 1], F32, kind="ExternalInput")
    bk_d = nc.dram_tensor("bk", [E, 1], F32, kind="ExternalInput")
    bvb_d = nc.dram_tensor("bvb", [128, E], BF16, kind="ExternalInput")
    onesc_d = nc.dram_tensor("onesc", [128, 4], BF16, kind="ExternalInput")
    outT = nc.dram_tensor("outT", [D, T], BF16, kind="ExternalOutput")
    outT_r = outT[:].rearrange("(g p) t -> p g t", p=128)  # [128, 8, T]
    # second partial for the last quarter's output projection (host-summed):
    # lets the d2=0 / d2=1 passes run as independent 1-matmul chains so the
    # tail is not serialized on the final normalize.
    outTb = nc.dram_tensor("outTb", [D, TQ], BF16, kind="ExternalOutput")
    outTb_r = outTb[:].rearrange("(g p) t -> p g t", p=128)  # [128, 8, TQ]

    with tile.TileContext(nc) as tc, nc.allow_low_precision(reason="bf16 attn"):
        with (
            tc.tile_pool(name="persist", bufs=1) as pp,
            tc.tile_pool(name="xw", bufs=1) as xw,
            tc.tile_pool(name="work", bufs=8) as wk_pool,
            tc.tile_pool(name="ostage", bufs=4) as op_pool,
            tc.tile_pool(name="small", bufs=3) as sm,
            tc.tile_pool(name="psum", bufs=3, space="PSUM") as ps,
            tc.tile_pool(name="psumy", bufs=2, space="PSUM") as psy,
        ):
            # ---- input DMAs.  The chase order matters: projection chunk c
            # can only run once x[c] + w[c] landed, so spread issue over three
            # sequencers (SP: x, ACT: wq/wk, Pool: wv) and keep per-chunk
            # order aligned with the c-major projection loops.
            xT_sb = [xw.tile([128, T], BF16, tag=f"x{c}", name=f"x{c}")
                     for c in range(DM)]
            # one DMA per weight tensor (a sequencer issues one DMA per
            # ~630ns; per-chunk weight DMAs were pacing the startup)
            wq_all = xw.tile([128, DM, E], BF16, tag="wq", name="wq")
            wk_all = xw.tile([128, DM, E], BF16, tag="wk", name="wk")
            wv_all = xw.tile([128, DM, E], BF16, tag="wv", name="wv")
            wq_sb = [wq_all[:, c, :] for c in range(DM)]
            wk_sb = [wk_all[:, c, :] for c in range(DM)]
            wv_sb = [wv_all[:, c, :] for c in range(DM)]
            wo_sb = [pp.tile([128, D], BF16, tag=f"wo{d2}", name=f"wo{d2}")
                     for d2 in range(2)]
            # halved weight DMAs ordered by first use (q chains consume wq
            # c0..7 first, then v chains wv, then k chains wk): the first
            # halves land well before the x-stream crowds the DMA device
            def w_half(dst, src, h):
                nc.scalar.dma_start(
                    out=dst[:, 4 * h:4 * (h + 1), :],
                    in_=src[4 * h * 128:4 * (h + 1) * 128, :].rearrange(
                        "(c p) e -> p c e", p=128))

            w_half(wq_all, wq, 0)
            w_half(wq_all, wq, 1)
            w_half(wv_all, wv, 0)
            w_half(wk_all, wk, 0)
            w_half(wv_all, wv, 1)
            w_half(wk_all, wk, 1)
            for c in range(DM):
                nc.sync.dma_start(
                    out=xT_sb[c][:, 0:TQ], in_=xT[c * 128:(c + 1) * 128, 0:TQ])
            bvb = pp.tile([128, E], BF16, tag="bvb")
            nc.sync.dma_start(out=bvb[:], in_=bvb_d[:, :])
            bq_sb, bk_sb = [], []
            for e2 in range(2):
                t_ = pp.tile([128, 1], F32, tag=f"bq{e2}")
                nc.sync.dma_start(out=t_[:], in_=bq_d[e2 * 128:(e2 + 1) * 128, :])
                bq_sb.append(t_)
                t_ = pp.tile([128, 1], F32, tag=f"bk{e2}")
                nc.sync.dma_start(out=t_[:], in_=bk_d[e2 * 128:(e2 + 1) * 128, :])
                bk_sb.append(t_)
            onesc = pp.tile([128, 4], BF16, tag="onesc")
            nc.sync.dma_start(out=onesc[:], in_=onesc_d[:, :])
            for c in range(DM):
                nc.sync.dma_start(
                    out=xT_sb[c][:, TQ:2 * TQ], in_=xT[c * 128:(c + 1) * 128, TQ:2 * TQ])
            for d2 in range(2):
                nc.scalar.dma_start(out=wo_sb[d2][:], in_=wo[d2 * 128:(d2 + 1) * 128, :])
            for c in range(DM):
                nc.sync.dma_start(
                    out=xT_sb[c][:, 2 * TQ:4 * TQ],
                    in_=xT[c * 128:(c + 1) * 128, 2 * TQ:4 * TQ])

            # all persistent activations are split into per-quarter tiles:
            # next-quarter projection writes and current-quarter attention /
            # output-projection reads then touch disjoint TILES, so the Tile
            # dependency tracker can never serialize a phase boundary on a
            # false write-vs-read conflict within one big tile.
            qT_sb = [[pp.tile([128, TQ], BF16, tag=f"qT{i}_{q}", name=f"qT{i}_{q}")
                      for q in range(NTQ)] for i in range(2)]
            # K stored per head in zero-padded 128-partition tiles: head a in
            # rows 0-63 (rows 64-127 zero), head b in rows 64-127 (0-63
            # zero).  The S matmuls then contract over all 128 q-dims with
            # the full combined q tile as the moving operand -- exact (zeros
            # kill the other head's contribution), same cost (matmul time =
            # moving columns), and every matmul in the kernel keeps tile_size
            # (128,128): no TensorE tiling-mode-switch drains between the S
            # (was 64-row mode) and y (128-row) matmuls of every k-block.
            kA_sb = [[pp.tile([128, TQ], BF16, tag=f"kA{i}_{q}", name=f"kA{i}_{q}")
                      for q in range(NTQ)] for i in range(2)]
            kB_sb = [[pp.tile([128, TQ], BF16, tag=f"kB{i}_{q}", name=f"kB{i}_{q}")
                      for q in range(NTQ)] for i in range(2)]
            for i in range(2):
                for q in range(NTQ):
                    nc.gpsimd.memset(kA_sb[i][q][64:128, :], 0.0)
                    nc.gpsimd.memset(kB_sb[i][q][0:64, :], 0.0)
            v_sb = [pp.tile([128, 4, HD + 1], BF16, tag=f"v{t}", name=f"v{t}")
                    for t in range(NTKT)]
            yT_sb = [[pp.tile([128, TQ], BF16, tag=f"yT{i}_{q}", name=f"yT{i}_{q}")
                      for q in range(NTQ)] for i in range(2)]

            # ---- projection half-phase: one of q/k (both e2 cols) + 2 v
            # t-blocks, c-major so each chunk's DMA unblocks its matmuls.
            def proj_half(tq, half):
                w_sb, b_sb, dst = ((wq_sb, bq_sb, qT_sb) if half == 0
                                   else (wk_sb, bk_sb, None))
                pqk = ps.tile([128, 1024], F32, tag="S", name=f"pp_{tq}_{half}")
                # two v chains in separate banks of an S-shaped tile (two
                # accumulation chains must not share a PSUM bank)
                pv = ps.tile([128, 1024], F32, tag="S", name=f"pv_{tq}_{half}")
                t0 = 4 * tq + 2 * half
                # at tq==0 the wv transfer is still in flight behind wq on the
                # weight queue: emit the qk chains first so the in-order
                # tensor queue isn't blocked by a v matmul waiting on wv
                phases = ([(0, 2), (1, 3)] if tq == 0 else [(0, 1, 2, 3)])
                for ops in phases:
                    for c in range(DM):
                        st, sp = (c == 0), (c == DM - 1)
                        for op in ops:
                            if op == 0:
                                nc.tensor.matmul(
                                    pqk[:, 0:TQ], w_sb[c][:, 0:128],
                                    xT_sb[c][:, tq * TQ:(tq + 1) * TQ],
                                    start=st, stop=sp)
                            elif op == 2:
                                nc.tensor.matmul(
                                    pqk[:, TQ:2 * TQ], w_sb[c][:, 128:256],
                                    xT_sb[c][:, tq * TQ:(tq + 1) * TQ],
                                    start=st, stop=sp)
                            elif op == 1:
                                nc.tensor.matmul(
                                    pv[:, 0:E],
                                    xT_sb[c][:, t0 * 128:(t0 + 1) * 128],
                                    wv_sb[c][:], start=st, stop=sp)
                            else:
                                nc.tensor.matmul(
                                    pv[:, TQ:TQ + E],
                                    xT_sb[c][:, (t0 + 1) * 128:(t0 + 2) * 128],
                                    wv_sb[c][:], start=st, stop=sp)
                # q/k bias adds run on ACT (Identity is in the same act table
                # as Exp, so no table reloads); keeping them off DVE drains
                # the DVE queue faster at phase boundaries, where the
                # counter-semaphore targets otherwise serialize behind it.
                for e2 in range(2):
                    if half == 0:
                        nc.scalar.activation(
                            dst[e2][tq][:, :], pqk[:, e2 * TQ:(e2 + 1) * TQ],
                            AF.Identity, bias=b_sb[e2][:])
                    else:
                        # split the k bias-add per head into the zero-padded
                        # per-head K tiles
                        nc.scalar.activation(
                            kA_sb[e2][tq][0:64, :],
                            pqk[0:64, e2 * TQ:(e2 + 1) * TQ],
                            AF.Identity, bias=b_sb[e2][0:64, :])
                        nc.scalar.activation(
                            kB_sb[e2][tq][64:128, :],
                            pqk[64:128, e2 * TQ:(e2 + 1) * TQ],
                            AF.Identity, bias=b_sb[e2][64:128, :])
                for j in range(2):
                    t = t0 + j
                    nc.vector.tensor_add(
                        out=v_sb[t][:, :, 0:HD],
                        in0=pv[:, j * TQ:j * TQ + E].rearrange("p (h d) -> p h d", h=4),
                        in1=bvb[:].rearrange("p (h d) -> p h d", h=4))
                    nc.vector.tensor_copy(
                        out=v_sb[t][:, :, HD:HD + 1],
                        in_=onesc[:].rearrange("p (h o) -> p h o", o=1))

            # ---- output projection for one e8 pair: 2-matmul chains staged
            # to SBUF (GpSimd copies; ACT is saturated by exp), one DMA per
            # pair.  With d2s=(0,)/(1,) only half the contraction is done and
            # the pair goes to `dst` as a 1-matmul chain (used to
            # de-serialize the tq3 tail; host sums partials).
            def oproj_pair(tq_o, pg, d2s=(0, 1), dst=None, tag=""):
                # the host-summed tq3 second partial travels in bf16 to halve
                # the tail DMA (its quantization only touches one quarter)
                ot = op_pool.tile([128, 2, TQ], BF16,
                                  tag="ostageb" if tag == "b" else "ostage",
                                  name=f"ot_{tq_o}_{pg}{tag}")
                pt = ps.tile([128, 1024], F32, tag="S",
                             name=f"poc_{tq_o}_{pg}{tag}")
                for j in range(2):
                    e8 = 2 * pg + j
                    for i, d2 in enumerate(d2s):
                        nc.tensor.matmul(
                            pt[:, j * TQ:(j + 1) * TQ],
                            wo_sb[d2][:, e8 * 128:(e8 + 1) * 128],
                            yT_sb[d2][tq_o][:, :],
                            start=(i == 0), stop=(i == len(d2s) - 1))
                    # staging PSUM->SBUF: only ACT/DVE can read PSUM.  The
                    # mid-attention-injected o3a pairs alternate ACT/DVE so
                    # neither engine's exp stream stalls; fill-slot pairs run
                    # on ACT (it only carries half the exp now).
                    if tag == "a" and pg % 2 == 1:
                        nc.vector.tensor_copy(out=ot[:, j, :],
                                              in_=pt[:, j * TQ:(j + 1) * TQ])
                    else:
                        nc.scalar.copy(out=ot[:, j, :],
                                       in_=pt[:, j * TQ:(j + 1) * TQ])
                if dst is None:
                    nc.sync.dma_start(
                        out=outT_r[:, 2 * pg:2 * (pg + 1),
                                   tq_o * TQ:(tq_o + 1) * TQ],
                        in_=ot[:])
                else:
                    nc.sync.dma_start(
                        out=dst[:, 2 * pg:2 * (pg + 1), :], in_=ot[:])

            # ---- one k-block of scores + exp for phase (tq, pr): S pair,
            # split exp (ACT real / DVE Schraudolph), causal mask on diag.
            # Standalone so a phase's tk=0 can be pre-emitted at the end of
            # the previous phase (the exp pipeline warms during the fill
            # instead of stalling each phase start).
            def s_stage_g(tq, pr, tk):
                ka = kA_sb[pr][tk // 4]
                kb = kB_sb[pr][tk // 4]
                qt = qT_sb[pr][tq]
                kc = (tk % 4) * 128
                # diag tiles only need columns >= 128*o (o = tk - 4*tq)
                o = tk - 4 * tq
                c0 = 128 * o if o > 0 else 0
                n = TQ - c0
                ps_s = ps.tile([128, 1024], F32, tag="S",
                               name=f"ps_s_{tq}_{pr}_{tk}")
                nc.tensor.matmul(
                    ps_s[:, c0:TQ],
                    ka[:, kc:kc + 128],
                    qt[:, c0:TQ],
                    start=True, stop=True)
                nc.tensor.matmul(
                    ps_s[:, TQ + c0:2 * TQ],
                    kb[:, kc:kc + 128],
                    qt[:, c0:TQ],
                    start=True, stop=True)
                # head a: real exp on ACT -> bf16
                es_a = wk_pool.tile([128, TQ], BF16, tag="esa",
                                    name=f"esa_{tq}_{pr}_{tk}")
                nc.scalar.activation(es_a[:, c0:TQ], ps_s[:, c0:TQ],
                                     AF.Exp, scale=0.125)
                # head b: Schraudolph exp on DVE -> f32 whose low 16 bits
                # are the bf16 pattern of exp(0.125*S)
                es_b = wk_pool.tile([128, TQ], F32, tag="esb",
                                    name=f"esb_{tq}_{pr}_{tk}")
                nc.vector.tensor_scalar(
                    out=es_b[:, c0:TQ], in0=ps_s[:, TQ + c0:2 * TQ],
                    scalar1=EXP_ALPHA, scalar2=EXP_MAGIC,
                    op0=ALU.mult, op1=ALU.add)
                if o >= 0:
                    em_a = wk_pool.tile([128, TQ], BF16, tag="esa",
                                        name=f"ema_{tq}_{pr}_{tk}")
                    nc.gpsimd.affine_select(
                        out=em_a[:, c0:TQ], in_=es_a[:, c0:TQ],
                        compare_op=mybir.AluOpType.is_ge,
                        fill=0.0, base=0, pattern=[[1, n]],
                        channel_multiplier=-1)
                    em_b = wk_pool.tile([128, TQ], F32, tag="esb",
                                        name=f"emb_{tq}_{pr}_{tk}")
                    nc.gpsimd.affine_select(
                        out=em_b[:, c0:TQ], in_=es_b[:, c0:TQ],
                        compare_op=mybir.AluOpType.is_ge,
                        fill=0.0, base=0, pattern=[[1, n]],
                        channel_multiplier=-1)
                    es_a, es_b = em_a, em_b
                return es_a, es_b, c0

            # ---- attention for one head pair (pr); returns a `finish`
            # closure (normalize muls) emitted later in a fill slot so the
            # tensor engine never waits on the reciprocal, plus the
            # pre-emitted first s-stage of the successor phase `next_`.
            def attention_pr(tq, pr, inject_at=None, inject=None,
                             pre=None, next_=None):
                ntk = 4 * (tq + 1)
                py_a = psy.tile([HD + 1, TQ], F32, tag="y", name=f"pya_{tq}_{pr}")
                py_b = psy.tile([HD + 1, TQ], F32, tag="y", name=f"pyb_{tq}_{pr}")

                def s_stage(tk):
                    return s_stage_g(tq, pr, tk)

                def y_stage(tk, es_a, es_b, c0):
                    nc.tensor.matmul(
                        py_a[:, c0:TQ], v_sb[tk][:, 2 * pr, :],
                        es_a[:, c0:TQ],
                        start=(tk == 0), stop=(tk == ntk - 1))
                    # read the f32 magic words' low halves as bf16 (stride 2)
                    es_b_bf = es_b[:].bitcast(BF16).rearrange(
                        "p (n two) -> p two n", two=2)[:, 0, :]
                    nc.tensor.matmul(
                        py_b[:, c0:TQ], v_sb[tk][:, 2 * pr + 1, :],
                        es_b_bf[:, c0:TQ],
                        start=(tk == 0), stop=(tk == ntk - 1))

                # depth-2 software pipeline: two s-stages are emitted before
                # the first (py-gated) y-stage, so at each phase start the PE
                # has independent score matmuls to run while the previous
                # phase's normalize chain releases the py accumulators.
                DEPTH = 2
                buf = [(0, pre if pre is not None else s_stage(0))]
                for tk in range(1, ntk):
                    if tk == inject_at:
                        inject()
                    buf.append((tk, s_stage(tk)))
                    if len(buf) > DEPTH:
                        t0, es = buf.pop(0)
                        y_stage(t0, *es)
                for t0, es in buf:
                    y_stage(t0, *es)
                nxt = s_stage_g(*next_, 0) if next_ is not None else None

                # custom DVE ops and partition_broadcast only work with
                # base-partition-0 operands (HW-probed: shifted bases read
                # the wrong partitions), so everything routes through
                # tile-rooted rows with native copies for the shifts.
                bc_a = sm.tile([64, TQ], F32, tag="bca", name=f"bca_{tq}_{pr}")
                bc_b = sm.tile([64, TQ], F32, tag="bcb", name=f"bcb_{tq}_{pr}")

                def emit_recip():
                    # chain py -> copy -> recip -> broadcast -> mul is on the
                    # phase-transition critical path: dn_a copies on ACT (free
                    # at phase end) in parallel with dn_b on DVE, each head's
                    # broadcast starts right after its own reciprocal, and the
                    # whole chain runs at high priority so it lands early in
                    # each engine's queue (the PE's counter-based semaphore
                    # waits otherwise serialize behind it).
                    with tc.high_priority():
                        dn_a = sm.tile([1, TQ], F32, tag="dna", name=f"dna_{tq}_{pr}")
                        dn_b = sm.tile([1, TQ], F32, tag="dnb", name=f"dnb_{tq}_{pr}")
                        nc.scalar.copy(out=dn_a[0:1, :], in_=py_a[HD:HD + 1, :])
                        nc.vector.tensor_copy(out=dn_b[0:1, :], in_=py_b[HD:HD + 1, :])
                        rc_a = sm.tile([1, TQ], F32, tag="rca", name=f"rca_{tq}_{pr}")
                        rc_b = sm.tile([1, TQ], F32, tag="rcb", name=f"rcb_{tq}_{pr}")
                        nc.vector.reciprocal_approx_fast(out=rc_a[0:1, :], in_=dn_a[0:1, :])
                        nc.gpsimd.partition_broadcast(
                            out_ap=bc_a[:, :], in_ap=rc_a[0:1, :])
                        nc.vector.reciprocal_approx_fast(out=rc_b[0:1, :], in_=dn_b[0:1, :])
                        nc.gpsimd.partition_broadcast(
                            out_ap=bc_b[:, :], in_ap=rc_b[0:1, :])

                def finish():
                    with tc.high_priority():
                        nc.vector.tensor_mul(
                            out=yT_sb[pr][tq][0:64, :],
                            in0=py_a[0:HD, :], in1=bc_a[:, :])
                        nc.vector.tensor_mul(
                            out=yT_sb[pr][tq][64:128, :],
                            in0=py_b[0:HD, :], in1=bc_b[:, :])

                emit_recip()
                return finish, nxt

            # ---- main schedule -------------------------------------------
            # per tq: [attn pr0][fill A][attn pr1][fill B]
            #   fill A: proj_half(tq+1, 0) + O(tq-1) pairs 0,1 + finish(pr0)
            #   fill B: proj_half(tq+1, 1) + O(tq-1) pairs 2,3 + finish(pr1)
            proj_half(0, 0)
            proj_half(0, 1)
            pre = None
            for tq in range(NTQ):
                fin0, pre = attention_pr(tq, 0, pre=pre, next_=(tq, 1))
                # tq1's A-projection is pulled into tq0's fill B (below), so
                # the thin tq0->tq1 boundary has extra late-emitted PE work
                # to cover the normalize chain
                if tq + 1 < NTQ and tq != 1:
                    proj_half(tq + 1, 0)
                if tq > 0:
                    oproj_pair(tq - 1, 0)
                    oproj_pair(tq - 1, 1)
                fin0()

                def o3a():
                    # d2=0 half of the last quarter's O-projection only needs
                    # yT[0] (normalized in the slot above); injected mid-pr1
                    # where the exp-paced attention has tensor slack, so it
                    # comes off the serial tail.
                    for pg in range(4):
                        oproj_pair(NTQ - 1, pg, d2s=(0,), tag="a")

                fin1, pre = attention_pr(
                    tq, 1,
                    inject_at=8 if tq == NTQ - 1 else None,
                    inject=o3a if tq == NTQ - 1 else None,
                    pre=pre,
                    next_=(tq + 1, 0) if tq + 1 < NTQ else None)
                if tq + 1 < NTQ:
                    proj_half(tq + 1, 1)
                if tq == 0:
                    # schedule-time floor stops the scheduler from hoisting
                    # this surplus projection into tq0's exp-paced attention
                    # slack -- it must stay available to cover the ~5.5us
                    # tq0->tq1 normalize-chain window (HW ~37us)
                    with tc.tile_wait_until(0.031):
                        proj_half(2, 0)
                if tq > 0 and tq < NTQ - 1:
                    oproj_pair(tq - 1, 2)
                    oproj_pair(tq - 1, 3)
                fin1()
            # the tq2 pair-2/3 O-projections are held back to the tail (with
            # a schedule-time floor so the scheduler cannot hoist them into
            # the attention phases): they are the only fin-independent PE
            # work left to cover the last normalize chain, which otherwise
            # leaves a ~2.4us PE gap and drops the clock to K=4/8 for the
            # final output projections.
            with tc.tile_wait_until(0.150):
                oproj_pair(NTQ - 2, 2)
                oproj_pair(NTQ - 2, 3)
            for pg in range(4):
                oproj_pair(NTQ - 1, pg, d2s=(1,), dst=outTb_r, tag="b")

    nc.compile()
    return nc


def _get_nc():
    if 'nc' not in _CACHE:
        _CACHE['nc'] = _build()
    return _CACHE['nc']


def _make_in_maps(x, Wq, bq, Wk, bk, Wv, bv, Wo, bo):
    import ml_dtypes
    BF = ml_dtypes.bfloat16
    x = np.asarray(x, dtype=np.float32)
    Wq = np.asarray(Wq, dtype=np.float32)
    Wk = np.asarray(Wk, dtype=np.float32)
    Wv = np.asarray(Wv, dtype=np.float32)
    Wo = np.asarray(Wo, dtype=np.float32)
    bq = np.asarray(bq, dtype=np.float32)
    bk = np.asarray(bk, dtype=np.float32)
    bv = np.asarray(bv, dtype=np.float32)

    onesc = np.ones((128, 4), dtype=BF)

    in_maps = []
    for c in range(NCORES):
        b, g = divmod(c, 4)
        hs = slice(g * E, (g + 1) * E)
        in_maps.append({
            "xT": np.ascontiguousarray(x[b].T).astype(BF),
            "wq": np.ascontiguousarray(Wq[hs].T).astype(BF),
            "wk": np.ascontiguousarray(Wk[hs].T).astype(BF),
            "wv": np.ascontiguousarray(Wv[hs].T).astype(BF),
            "wo": np.ascontiguousarray(Wo[:, hs].T).astype(BF),
            "bq": np.ascontiguousarray(bq[hs].reshape(E, 1)),
            "bk": np.ascontiguousarray(bk[hs].reshape(E, 1)),
            "bvb": np.broadcast_to(bv[hs], (128, E)).astype(BF),
            "onesc": onesc,
        })
    return in_maps


def kernel(x, Wq, bq, Wk, bk, Wv, bv, Wo, bo, _run_kwargs=None):
    nc = _get_nc()
    in_maps = _make_in_maps(x, Wq, bq, Wk, bk, Wv, bv, Wo, bo)
    last_err = None
    for _attempt in range(3):
        try:
            res = run_bass_kernel_spmd(nc, in_maps, core_ids=list(range(NCORES)),
                                       **(_run_kwargs or {}))
            break
        except Exception as e:  # transient NRT/device hiccups: retry
            last_err = e
            import time as _time
            _time.sleep(2.0)
    else:
        raise last_err
    bo = np.asarray(bo, dtype=np.float32)
    out = np.empty((B, T, D), dtype=np.float32)
    for b in range(B):
        acc = res.results[4 * b]["outT"].astype(np.float32)
        acc[:, 3 * TQ:4 * TQ] += res.results[4 * b]["outTb"].astype(np.float32)
        for g in range(1, 4):
            acc += res.results[4 * b + g]["outT"].astype(np.float32)
            acc[:, 3 * TQ:4 * TQ] += res.results[4 * b + g]["outTb"].astype(np.float32)
        out[b] = acc.T + bo
    if _run_kwargs:
        _CACHE['last_results'] = res
    return out


# revision 43
# speedup vs baseline: 1.0007x; 1.0007x over previous
"""Causal self-attention (B=2, T=2048, D=1024, H=16) on 8 TRN2 NeuronCores.

Sharding: data-parallel over batch (cores 0-3 -> batch 0, cores 4-7 -> batch 1),
tensor-parallel over heads (4 heads / 256 output dims per core). Each core
computes q/k/v projections for its heads, causal flash-style attention, and a
partial output projection (contraction over its 256 dims of Wo). The host sums
the 4 partials per batch and adds bo.

All matmul operands are bf16 (f32 PSUM accumulation): weight loads get FWL
(2x) and x/weight DMAs halve. The softmax exp is split across two engines:
head a of each pair runs real exp on ACT; head b runs a one-instruction
Schraudolph exp on DVE (w = alpha*S + (beta + 1.5*2^23) in f32 -- the float
add's round-to-nearest leaves round(alpha*S+beta) in the low mantissa bits,
which the PE reads as bf16 via a stride-2 bitcast view).  This halves the
exp time per k-block that paces the attention inner loop.  Softmax
normalization uses reciprocal_approx_fast straight from PSUM and a stride-0
partition-broadcast DMA instead of PE broadcast matmuls.  Output-projection
PSUM->SBUF staging runs on GpSimd so ACT only does exp.  DMA issue is spread
across the SP/ACT/Pool sequencers; outputs are staged in pairs and stored
with one DMA per 2 row-blocks.
"""
import sys

sys.path.insert(0, '/opt/trn_rl_repo')

import numpy as np

import concourse.bass as bass  # noqa: F401  (import keeps bass registered)
import concourse.mybir as mybir
import concourse.tile as tile
from concourse import bacc
from concourse.bass_utils import run_bass_kernel_spmd

F32 = mybir.dt.float32
BF16 = mybir.dt.bfloat16
AF = mybir.ActivationFunctionType
ALU = mybir.AluOpType

B, T, D, H, HD = 2, 2048, 1024, 16, 64
NCORES = 8
E = 256          # output dims per core (4 heads x 64)
DM = 8           # d_model chunks of 128
TQ = 512
NTQ = T // TQ    # 4
TKT = 128
NTKT = T // TKT  # 16

# Schraudolph exp constants: bf16 bits of exp(0.125*S) = round(ALPHA*S + BETA)
# computed via the f32 magic-add (1.5*2^23) round-to-nearest trick.
EXP_ALPHA = 0.125 * 1.4426950408889634 * 128.0    # 23.08312065422342
EXP_MAGIC = float((127 * 128 - 6) + (3 << 22))    # beta + 1.5*2^23

_CACHE = {}


def _build():
    nc = bacc.Bacc("TRN2", target_bir_lowering=False, debug=False)

    xT = nc.dram_tensor("xT", [D, T], BF16, kind="ExternalInput")
    wq = nc.dram_tensor("wq", [D, E], BF16, kind="ExternalInput")
    wk = nc.dram_tensor("wk", [D, E], BF16, kind="ExternalInput")
    wv = nc.dram_tensor("wv", [D, E], BF16, kind="ExternalInput")
    wo = nc.dram_tensor("wo", [E, D], BF16, kind="ExternalInput")
    bq_d = nc.dram_tensor("bq", [E, 1], F32, kind="ExternalInput")
    bk_d = nc.dram_tensor("bk", [E, 1], F32, kind="ExternalInput")
    bvb_d = nc.dram_tensor("bvb", [128, E], BF16, kind="ExternalInput")
    onesc_d = nc.dram_tensor("onesc", [128, 4], BF16, kind="ExternalInput")
    outT = nc.dram_tensor("outT", [D, T], BF16, kind="ExternalOutput")
    outT_r = outT[:].rearrange("(g p) t -> p g t", p=128)  # [128, 8, T]
    # second partial for the last quarter's output projection (host-summed):
    # lets the d2=0 / d2=1 passes run as independent 1-matmul chains so the
    # tail is not serialized on the final normalize.
    outTb = nc.dram_tensor("outTb", [D, TQ], BF16, kind="ExternalOutput")
    outTb_r = outTb[:].rearrange("(g p) t -> p g t", p=128)  # [128, 8, TQ]

    with tile.TileContext(nc) as tc, nc.allow_low_precision(reason="bf16 attn"):
        with (
            tc.tile_pool(name="persist", bufs=1) as pp,
            tc.tile_pool(name="xw", bufs=1) as xw,
            tc.tile_pool(name="work", bufs=8) as wk_pool,
            tc.tile_pool(name="ostage", bufs=4) as op_pool,
            tc.tile_pool(name="small", bufs=3) as sm,
            tc.tile_pool(name="psum", bufs=3, space="PSUM") as ps,
            tc.tile_pool(name="psumy", bufs=2, space="PSUM") as psy,
        ):
            # ---- input DMAs.  The chase order matters: projection chunk c
            # can only run once x[c] + w[c] landed, so spread issue over three
            # sequencers (SP: x, ACT: wq/wk, Pool: wv) and keep per-chunk
            # order aligned with the c-major projection loops.
            xT_sb = [xw.tile([128, T], BF16, tag=f"x{c}", name=f"x{c}")
                     for c in range(DM)]
            # one DMA per weight tensor (a sequencer issues one DMA per
            # ~630ns; per-chunk weight DMAs were pacing the startup)
            wq_all = xw.tile([128, DM, E], BF16, tag="wq", name="wq")
            wk_all = xw.tile([128, DM, E], BF16, tag="wk", name="wk")
            wv_all = xw.tile([128, DM, E], BF16, tag="wv", name="wv")
            wq_sb = [wq_all[:, c, :] for c in range(DM)]
            wk_sb = [wk_all[:, c, :] for c in range(DM)]
            wv_sb = [wv_all[:, c, :] for c in range(DM)]
            wo_sb = [pp.tile([128, D], BF16, tag=f"wo{d2}", name=f"wo{d2}")
                     for d2 in range(2)]
            # halved weight DMAs ordered by first use (q chains consume wq
            # c0..7 first, then v chains wv, then k chains wk): the first
            # halves land well before the x-stream crowds the DMA device
            def w_half(dst, src, h):
                nc.scalar.dma_start(
                    out=dst[:, 4 * h:4 * (h + 1), :],
                    in_=src[4 * h * 128:4 * (h + 1) * 128, :].rearrange(
                        "(c p) e -> p c e", p=128))

            w_half(wq_all, wq, 0)
            w_half(wq_all, wq, 1)
            w_half(wv_all, wv, 0)
            w_half(wk_all, wk, 0)
            w_half(wv_all, wv, 1)
            w_half(wk_all, wk, 1)
            for c in range(DM):
                nc.sync.dma_start(
                    out=xT_sb[c][:, 0:TQ], in_=xT[c * 128:(c + 1) * 128, 0:TQ])
            bvb = pp.tile([128, E], BF16, tag="bvb")
            nc.sync.dma_start(out=bvb[:], in_=bvb_d[:, :])
            bq_sb, bk_sb = [], []
            for e2 in range(2):
                t_ = pp.tile([128, 1], F32, tag=f"bq{e2}")
                nc.sync.dma_start(out=t_[:], in_=bq_d[e2 * 128:(e2 + 1) * 128, :])
                bq_sb.append(t_)
                t_ = pp.tile([128, 1], F32, tag=f"bk{e2}")
                nc.sync.dma_start(out=t_[:], in_=bk_d[e2 * 128:(e2 + 1) * 128, :])
                bk_sb.append(t_)
            onesc = pp.tile([128, 4], BF16, tag="onesc")
            nc.sync.dma_start(out=onesc[:], in_=onesc_d[:, :])
            for c in range(DM):
                nc.sync.dma_start(
                    out=xT_sb[c][:, TQ:2 * TQ], in_=xT[c * 128:(c + 1) * 128, TQ:2 * TQ])
            for d2 in range(2):
                nc.scalar.dma_start(out=wo_sb[d2][:], in_=wo[d2 * 128:(d2 + 1) * 128, :])
            for c in range(DM):
                nc.sync.dma_start(
                    out=xT_sb[c][:, 2 * TQ:4 * TQ],
                    in_=xT[c * 128:(c + 1) * 128, 2 * TQ:4 * TQ])

            # all persistent activations are split into per-quarter tiles:
            # next-quarter projection writes and current-quarter attention /
            # output-projection reads then touch disjoint TILES, so the Tile
            # dependency tracker can never serialize a phase boundary on a
            # false write-vs-read conflict within one big tile.
            qT_sb = [[pp.tile([128, TQ], BF16, tag=f"qT{i}_{q}", name=f"qT{i}_{q}")
                      for q in range(NTQ)] for i in range(2)]
            # K stored per head in zero-padded 128-partition tiles: head a in
            # rows 0-63 (rows 64-127 zero), head b in rows 64-127 (0-63
            # zero).  The S matmuls then contract over all 128 q-dims with
            # the full combined q tile as the moving operand -- exact (zeros
            # kill the other head's contribution), same cost (matmul time =
            # moving columns), and every matmul in the kernel keeps tile_size
            # (128,128): no TensorE tiling-mode-switch drains between the S
            # (was 64-row mode) and y (128-row) matmuls of every k-block.
            kA_sb = [[pp.tile([128, TQ], BF16, tag=f"kA{i}_{q}", name=f"kA{i}_{q}")
                      for q in range(NTQ)] for i in range(2)]
            kB_sb = [[pp.tile([128, TQ], BF16, tag=f"kB{i}_{q}", name=f"kB{i}_{q}")
                      for q in range(NTQ)] for i in range(2)]
            for i in range(2):
                for q in range(NTQ):
                    nc.gpsimd.memset(kA_sb[i][q][64:128, :], 0.0)
                    nc.gpsimd.memset(kB_sb[i][q][0:64, :], 0.0)
            v_sb = [pp.tile([128, 4, HD + 1], BF16, tag=f"v{t}", name=f"v{t}")
                    for t in range(NTKT)]
            yT_sb = [[pp.tile([128, TQ], BF16, tag=f"yT{i}_{q}", name=f"yT{i}_{q}")
                      for q in range(NTQ)] for i in range(2)]

            # ---- projection half-phase: one of q/k (both e2 cols) + 2 v
            # t-blocks, c-major so each chunk's DMA unblocks its matmuls.
            def proj_half(tq, half):
                w_sb, b_sb, dst = ((wq_sb, bq_sb, qT_sb) if half == 0
                                   else (wk_sb, bk_sb, None))
                pqk = ps.tile([128, 1024], F32, tag="S", name=f"pp_{tq}_{half}")
                # two v chains in separate banks of an S-shaped tile (two
                # accumulation chains must not share a PSUM bank)
                pv = ps.tile([128, 1024], F32, tag="S", name=f"pv_{tq}_{half}")
                t0 = 4 * tq + 2 * half
                # at tq==0 the wv transfer is still in flight behind wq on the
                # weight queue: emit the qk chains first so the in-order
                # tensor queue isn't blocked by a v matmul waiting on wv
                phases = ([(0, 2), (1, 3)] if tq == 0 else [(0, 1, 2, 3)])
                for ops in phases:
                    for c in range(DM):
                        st, sp = (c == 0), (c == DM - 1)
                        for op in ops:
                            if op == 0:
                                nc.tensor.matmul(
                                    pqk[:, 0:TQ], w_sb[c][:, 0:128],
                                    xT_sb[c][:, tq * TQ:(tq + 1) * TQ],
                                    start=st, stop=sp)
                            elif op == 2:
                                nc.tensor.matmul(
                                    pqk[:, TQ:2 * TQ], w_sb[c][:, 128:256],
                                    xT_sb[c][:, tq * TQ:(tq + 1) * TQ],
                                    start=st, stop=sp)
                            elif op == 1:
                                nc.tensor.matmul(
                                    pv[:, 0:E],
                                    xT_sb[c][:, t0 * 128:(t0 + 1) * 128],
                                    wv_sb[c][:], start=st, stop=sp)
                            else:
                                nc.tensor.matmul(
                                    pv[:, TQ:TQ + E],
                                    xT_sb[c][:, (t0 + 1) * 128:(t0 + 2) * 128],
                                    wv_sb[c][:], start=st, stop=sp)
                # q/k bias adds run on ACT (Identity is in the same act table
                # as Exp, so no table reloads); keeping them off DVE drains
                # the DVE queue faster at phase boundaries, where the
                # counter-semaphore targets otherwise serialize behind it.
                for e2 in range(2):
                    if half == 0:
                        nc.scalar.activation(
                            dst[e2][tq][:, :], pqk[:, e2 * TQ:(e2 + 1) * TQ],
                            AF.Identity, bias=b_sb[e2][:])
                    else:
                        # split the k bias-add per head into the zero-padded
                        # per-head K tiles
                        nc.scalar.activation(
                            kA_sb[e2][tq][0:64, :],
                            pqk[0:64, e2 * TQ:(e2 + 1) * TQ],
                            AF.Identity, bias=b_sb[e2][0:64, :])
                        nc.scalar.activation(
                            kB_sb[e2][tq][64:128, :],
                            pqk[64:128, e2 * TQ:(e2 + 1) * TQ],
                            AF.Identity, bias=b_sb[e2][64:128, :])
                for j in range(2):
                    t = t0 + j
                    nc.vector.tensor_add(
                        out=v_sb[t][:, :, 0:HD],
                        in0=pv[:, j * TQ:j * TQ + E].rearrange("p (h d) -> p h d", h=4),
                        in1=bvb[:].rearrange("p (h d) -> p h d", h=4))
                    nc.vector.tensor_copy(
                        out=v_sb[t][:, :, HD:HD + 1],
                        in_=onesc[:].rearrange("p (h o) -> p h o", o=1))

            # ---- output projection for one e8 pair: 2-matmul chains staged
            # to SBUF (GpSimd copies; ACT is saturated by exp), one DMA per
            # pair.  With d2s=(0,)/(1,) only half the contraction is done and
            # the pair goes to `dst` as a 1-matmul chain (used to
            # de-serialize the tq3 tail; host sums partials).
            def oproj_pair(tq_o, pg, d2s=(0, 1), dst=None, tag=""):
                # the host-summed tq3 second partial travels in bf16 to halve
                # the tail DMA (its quantization only touches one quarter)
                ot = op_pool.tile([128, 2, TQ], BF16,
                                  tag="ostageb" if tag == "b" else "ostage",
                                  name=f"ot_{tq_o}_{pg}{tag}")
                pt = ps.tile([128, 1024], F32, tag="S",
                             name=f"poc_{tq_o}_{pg}{tag}")
                for j in range(2):
                    e8 = 2 * pg + j
                    for i, d2 in enumerate(d2s):
                        nc.tensor.matmul(
                            pt[:, j * TQ:(j + 1) * TQ],
                            wo_sb[d2][:, e8 * 128:(e8 + 1) * 128],
                            yT_sb[d2][tq_o][:, :],
                            start=(i == 0), stop=(i == len(d2s) - 1))
                    # staging PSUM->SBUF: only ACT/DVE can read PSUM.  The
                    # mid-attention-injected o3a pairs alternate ACT/DVE so
                    # neither engine's exp stream stalls; fill-slot pairs run
                    # on ACT (it only carries half the exp now).
                    if tag == "a" and pg % 2 == 1:
                        nc.vector.tensor_copy(out=ot[:, j, :],
                                              in_=pt[:, j * TQ:(j + 1) * TQ])
                    else:
                        nc.scalar.copy(out=ot[:, j, :],
                                       in_=pt[:, j * TQ:(j + 1) * TQ])
                if dst is None:
                    nc.sync.dma_start(
                        out=outT_r[:, 2 * pg:2 * (pg + 1),
                                   tq_o * TQ:(tq_o + 1) * TQ],
                        in_=ot[:])
                else:
                    nc.sync.dma_start(
                        out=dst[:, 2 * pg:2 * (pg + 1), :], in_=ot[:])

            # ---- one k-block of scores + exp for phase (tq, pr): S pair,
            # split exp (ACT real / DVE Schraudolph), causal mask on diag.
            # Standalone so a phase's tk=0 can be pre-emitted at the end of
            # the previous phase (the exp pipeline warms during the fill
            # instead of stalling each phase start).
            def s_stage_g(tq, pr, tk):
                ka = kA_sb[pr][tk // 4]
                kb = kB_sb[pr][tk // 4]
                qt = qT_sb[pr][tq]
                kc = (tk % 4) * 128
                # diag tiles only need columns >= 128*o (o = tk - 4*tq)
                o = tk - 4 * tq
                c0 = 128 * o if o > 0 else 0
                n = TQ - c0
                ps_s = ps.tile([128, 1024], F32, tag="S",
                               name=f"ps_s_{tq}_{pr}_{tk}")
                nc.tensor.matmul(
                    ps_s[:, c0:TQ],
                    ka[:, kc:kc + 128],
                    qt[:, c0:TQ],
                    start=True, stop=True)
                nc.tensor.matmul(
                    ps_s[:, TQ + c0:2 * TQ],
                    kb[:, kc:kc + 128],
                    qt[:, c0:TQ],
                    start=True, stop=True)
                # head a: real exp on ACT -> bf16
                es_a = wk_pool.tile([128, TQ], BF16, tag="esa",
                                    name=f"esa_{tq}_{pr}_{tk}")
                nc.scalar.activation(es_a[:, c0:TQ], ps_s[:, c0:TQ],
                                     AF.Exp, scale=0.125)
                # head b: Schraudolph exp on DVE -> f32 whose low 16 bits
                # are the bf16 pattern of exp(0.125*S)
                es_b = wk_pool.tile([128, TQ], F32, tag="esb",
                                    name=f"esb_{tq}_{pr}_{tk}")
                nc.vector.tensor_scalar(
                    out=es_b[:, c0:TQ], in0=ps_s[:, TQ + c0:2 * TQ],
                    scalar1=EXP_ALPHA, scalar2=EXP_MAGIC,
                    op0=ALU.mult, op1=ALU.add)
                if o >= 0:
                    em_a = wk_pool.tile([128, TQ], BF16, tag="esa",
                                        name=f"ema_{tq}_{pr}_{tk}")
                    nc.gpsimd.affine_select(
                        out=em_a[:, c0:TQ], in_=es_a[:, c0:TQ],
                        compare_op=mybir.AluOpType.is_ge,
                        fill=0.0, base=0, pattern=[[1, n]],
                        channel_multiplier=-1)
                    em_b = wk_pool.tile([128, TQ], F32, tag="esb",
                                        name=f"emb_{tq}_{pr}_{tk}")
                    nc.gpsimd.affine_select(
                        out=em_b[:, c0:TQ], in_=es_b[:, c0:TQ],
                        compare_op=mybir.AluOpType.is_ge,
                        fill=0.0, base=0, pattern=[[1, n]],
                        channel_multiplier=-1)
                    es_a, es_b = em_a, em_b
                return es_a, es_b, c0

            # ---- attention for one head pair (pr); returns a `finish`
            # closure (normalize muls) emitted later in a fill slot so the
            # tensor engine never waits on the reciprocal, plus the
            # pre-emitted first s-stage of the successor phase `next_`.
            def attention_pr(tq, pr, inject_at=None, inject=None,
                             pre=None, next_=None):
                ntk = 4 * (tq + 1)
                py_a = psy.tile([HD + 1, TQ], F32, tag="y", name=f"pya_{tq}_{pr}")
                py_b = psy.tile([HD + 1, TQ], F32, tag="y", name=f"pyb_{tq}_{pr}")

                def s_stage(tk):
                    return s_stage_g(tq, pr, tk)

                def y_stage(tk, es_a, es_b, c0):
                    nc.tensor.matmul(
                        py_a[:, c0:TQ], v_sb[tk][:, 2 * pr, :],
                        es_a[:, c0:TQ],
                        start=(tk == 0), stop=(tk == ntk - 1))
                    # read the f32 magic words' low halves as bf16 (stride 2)
                    es_b_bf = es_b[:].bitcast(BF16).rearrange(
                        "p (n two) -> p two n", two=2)[:, 0, :]
                    nc.tensor.matmul(
                        py_b[:, c0:TQ], v_sb[tk][:, 2 * pr + 1, :],
                        es_b_bf[:, c0:TQ],
                        start=(tk == 0), stop=(tk == ntk - 1))

                # depth-2 software pipeline: two s-stages are emitted before
                # the first (py-gated) y-stage, so at each phase start the PE
                # has independent score matmuls to run while the previous
                # phase's normalize chain releases the py accumulators.
                DEPTH = 2
                buf = [(0, pre if pre is not None else s_stage(0))]
                for tk in range(1, ntk):
                    if tk == inject_at:
                        inject()
                    buf.append((tk, s_stage(tk)))
                    if len(buf) > DEPTH:
                        t0, es = buf.pop(0)
                        y_stage(t0, *es)
                for t0, es in buf:
                    y_stage(t0, *es)
                nxt = s_stage_g(*next_, 0) if next_ is not None else None

                # custom DVE ops and partition_broadcast only work with
                # base-partition-0 operands (HW-probed: shifted bases read
                # the wrong partitions), so everything routes through
                # tile-rooted rows with native copies for the shifts.
                bc_a = sm.tile([64, TQ], F32, tag="bca", name=f"bca_{tq}_{pr}")
                bc_b = sm.tile([64, TQ], F32, tag="bcb", name=f"bcb_{tq}_{pr}")

                def emit_recip():
                    # chain py -> copy -> recip -> broadcast -> mul is on the
                    # phase-transition critical path: dn_a copies on ACT (free
                    # at phase end) in parallel with dn_b on DVE, each head's
                    # broadcast starts right after its own reciprocal, and the
                    # whole chain runs at high priority so it lands early in
                    # each engine's queue (the PE's counter-based semaphore
                    # waits otherwise serialize behind it).
                    with tc.high_priority():
                        dn_a = sm.tile([1, TQ], F32, tag="dna", name=f"dna_{tq}_{pr}")
                        dn_b = sm.tile([1, TQ], F32, tag="dnb", name=f"dnb_{tq}_{pr}")
                        nc.scalar.copy(out=dn_a[0:1, :], in_=py_a[HD:HD + 1, :])
                        nc.vector.tensor_copy(out=dn_b[0:1, :], in_=py_b[HD:HD + 1, :])
                        rc_a = sm.tile([1, TQ], F32, tag="rca", name=f"rca_{tq}_{pr}")
                        rc_b = sm.tile([1, TQ], F32, tag="rcb", name=f"rcb_{tq}_{pr}")
                        nc.vector.reciprocal_approx_fast(out=rc_a[0:1, :], in_=dn_a[0:1, :])
                        nc.gpsimd.partition_broadcast(
                            out_ap=bc_a[:, :], in_ap=rc_a[0:1, :])
                        nc.vector.reciprocal_approx_fast(out=rc_b[0:1, :], in_=dn_b[0:1, :])
                        nc.gpsimd.partition_broadcast(
                            out_ap=bc_b[:, :], in_ap=rc_b[0:1, :])

                def finish():
                    with tc.high_priority():
                        nc.vector.tensor_mul(
                            out=yT_sb[pr][tq][0:64, :],
                            in0=py_a[0:HD, :], in1=bc_a[:, :])
                        nc.vector.tensor_mul(
                            out=yT_sb[pr][tq][64:128, :],
                            in0=py_b[0:HD, :], in1=bc_b[:, :])

                emit_recip()
                return finish, nxt

            # ---- main schedule -------------------------------------------
            # per tq: [attn pr0][fill A][attn pr1][fill B]
            #   fill A: proj_half(tq+1, 0) + O(tq-1) pairs 0,1 + finish(pr0)
            #   fill B: proj_half(tq+1, 1) + O(tq-1) pairs 2,3 + finish(pr1)
            proj_half(0, 0)
            proj_half(0, 1)
            pre = None
            for tq in range(NTQ):
                fin0, pre = attention_pr(tq, 0, pre=pre, next_=(tq, 1))
                # tq1's A-projection is pulled into tq0's fill B (below), so
                # the thin tq0->tq1 boundary has extra late-emitted PE work
                # to cover the normalize chain
                if tq + 1 < NTQ and tq != 1:
                    proj_half(tq + 1, 0)
                if tq > 0:
                    oproj_pair(tq - 1, 0)
                    oproj_pair(tq - 1, 1)
                fin0()

                def o3a():
                    # d2=0 half of the last quarter's O-projection only needs
                    # yT[0] (normalized in the slot above); injected mid-pr1
                    # where the exp-paced attention has tensor slack, so it
                    # comes off the serial tail.
                    for pg in range(4):
                        oproj_pair(NTQ - 1, pg, d2s=(0,), tag="a")

                fin1, pre = attention_pr(
                    tq, 1,
                    inject_at=8 if tq == NTQ - 1 else None,
                    inject=o3a if tq == NTQ - 1 else None,
                    pre=pre,
                    next_=(tq + 1, 0) if tq + 1 < NTQ else None)
                if tq + 1 < NTQ:
                    proj_half(tq + 1, 1)
                if tq == 0:
                    # dependency-based hold: add an exact zero (derived from
                    # the tq1-pr0 prologue's es tile, which is written at the
                    # very end of tq0-pr1's attention) into one x cell that
                    # proj(2,0)'s first q- and v-chain matmuls read.  The
                    # scheduler then cannot hoist this surplus projection
                    # into tq0's exp-paced attention slack, but it becomes
                    # ready exactly at the boundary where the normalize
                    # chain otherwise stalls the PE for ~5.6us.  (Time-based
                    # tile_wait_until floors could not be calibrated here:
                    # the scheduler's sim timeline is not exposed.)
                    aux = sm.tile([1, 1], BF16, tag="aux", name="aux_gate")
                    nc.vector.tensor_scalar_mul(
                        out=aux[0:1, :], in0=pre[0][0:1, 0:1], scalar1=0.0)
                    nc.vector.tensor_add(
                        out=xT_sb[0][0:1, 2 * TQ:2 * TQ + 1],
                        in0=xT_sb[0][0:1, 2 * TQ:2 * TQ + 1], in1=aux[0:1, :])
                    proj_half(2, 0)
                if tq > 0 and tq < NTQ - 1:
                    oproj_pair(tq - 1, 2)
                    oproj_pair(tq - 1, 3)
                fin1()
            # the tq2 pair-2/3 O-projections are held back to the tail (with
            # a schedule-time floor so the scheduler cannot hoist them into
            # the attention phases): they are the only fin-independent PE
            # work left to cover the last normalize chain, which otherwise
            # leaves a ~2.4us PE gap and drops the clock to K=4/8 for the
            # final output projections.
            with tc.tile_wait_until(0.150):
                oproj_pair(NTQ - 2, 2)
                oproj_pair(NTQ - 2, 3)
            for pg in range(4):
                oproj_pair(NTQ - 1, pg, d2s=(1,), dst=outTb_r, tag="b")

    nc.compile()
    return nc


def _get_nc():
    if 'nc' not in _CACHE:
        _CACHE['nc'] = _build()
    return _CACHE['nc']


def _make_in_maps(x, Wq, bq, Wk, bk, Wv, bv, Wo, bo):
    import ml_dtypes
    BF = ml_dtypes.bfloat16
    x = np.asarray(x, dtype=np.float32)
    Wq = np.asarray(Wq, dtype=np.float32)
    Wk = np.asarray(Wk, dtype=np.float32)
    Wv = np.asarray(Wv, dtype=np.float32)
    Wo = np.asarray(Wo, dtype=np.float32)
    bq = np.asarray(bq, dtype=np.float32)
    bk = np.asarray(bk, dtype=np.float32)
    bv = np.asarray(bv, dtype=np.float32)

    onesc = np.ones((128, 4), dtype=BF)

    in_maps = []
    for c in range(NCORES):
        b, g = divmod(c, 4)
        hs = slice(g * E, (g + 1) * E)
        in_maps.append({
            "xT": np.ascontiguousarray(x[b].T).astype(BF),
            "wq": np.ascontiguousarray(Wq[hs].T).astype(BF),
            "wk": np.ascontiguousarray(Wk[hs].T).astype(BF),
            "wv": np.ascontiguousarray(Wv[hs].T).astype(BF),
            "wo": np.ascontiguousarray(Wo[:, hs].T).astype(BF),
            "bq": np.ascontiguousarray(bq[hs].reshape(E, 1)),
            "bk": np.ascontiguousarray(bk[hs].reshape(E, 1)),
            "bvb": np.broadcast_to(bv[hs], (128, E)).astype(BF),
            "onesc": onesc,
        })
    return in_maps


def kernel(x, Wq, bq, Wk, bk, Wv, bv, Wo, bo, _run_kwargs=None):
    nc = _get_nc()
    in_maps = _make_in_maps(x, Wq, bq, Wk, bk, Wv, bv, Wo, bo)
    last_err = None
    for _attempt in range(3):
        try:
            res = run_bass_kernel_spmd(nc, in_maps, core_ids=list(range(NCORES)),
                                       **(_run_kwargs or {}))
            break
        except Exception as e:  # transient NRT/device hiccups: retry
            last_err = e
            import time as _time
            _time.sleep(2.0)
    else:
        raise last_err
    bo = np.asarray(bo, dtype=np.float32)
    out = np.empty((B, T, D), dtype=np.float32)
    for b in range(B):
        acc = res.results[4 * b]["outT"].astype(np.float32)
        acc[:, 3 * TQ:4 * TQ] += res.results[4 * b]["outTb"].astype(np.float32)
        for g in range(1, 4):
            acc += res.results[4 * b + g]["outT"].astype(np.float32)
            acc[:, 3 * TQ:4 * TQ] += res.results[4 * b + g]["outTb"].astype(np.float32)
        out[b] = acc.T + bo
    if _run_kwargs:
        _CACHE['last_results'] = res
    return out
